# revision 4
# baseline (speedup 1.0000x reference)
"""Trainium2 Bass kernel for nn_DCTExtractor.

Reference computation:
  - stego [8, 3, 1024, 1024] f32; per 8x8 block 2D DCT-II (D @ X @ D^T).
  - bits[i] = abs(round_half_even(dct[b,c,nh,nw,bh,bw])) % 2 for 1572864
    index tuples.
  - out [8, num_bits]: out[b_idx[i], i] = bits[i]; other rows 0.

Sharding: data-parallel over batch b across the 8 NeuronCores; core b
processes image b and produces output row b.

Fast path (canonical meshgrid indices): each needed coefficient is a
Frobenius inner product <X_block, D_bh (x) D_bw>.  The host re-lays each
image out with BOTH within-block axes on partitions:

    x[c, (q, j) 128, (g, nw, k) 8192]   with nh = g*16 + q, w = nw*8 + k

and splits it into a bf16 hi/lo pair (xa + xb == x to ~16 mantissa bits).
The whole per-block 2D DCT then becomes 8 PSUM-accumulated bf16 matmuls
per (channel, g-half) with CONSTANT stationaries

    Wk[(q,j), (q,plane)] = D[bh_p, j] * D[bw_p, k]   (block-diagonal in q)

run as 3 split terms (Wa@xa + Wa@xb + Wb@xa) for fp32-grade accuracy:
every pixel streams through the PE exactly 3 times at 1 cycle/row and the
coefficients land already organized [64 (q,plane), (g,nw)] for the DVE
parity chain (round-to-int via the 1.5*2^23 magic trick, all
intermediates exact).  Parity bits return as bf16 (exact for 0/1).

General fallback (arbitrary indices): device computes the full 64-plane
parity table per image; host gathers bits and applies the b mask.
"""

import sys

if "/opt/trn_rl_repo" not in sys.path:
    sys.path.insert(0, "/opt/trn_rl_repo")

import numpy as np
import ml_dtypes

BS = 8
B, C, H, W = 8, 3, 1024, 1024
NBH, NBW = H // BS, W // BS
POS = np.array([[1, 2], [2, 1], [2, 2], [3, 1]], dtype=np.int32)
NPOS = 4
SEG = C * NBH * NBW * NPOS  # bits per batch element = 196608
NUM_BITS = B * SEG
MAGIC = float(np.float32(8388608.0))  # 2^23 (general path, abs first)
M15 = float(np.float32(12582912.0))  # 1.5*2^23: RNE for signed values
IP = [0, 1, 1, 2]  # i' = bh-1 per p
LP = [2, 1, 2, 1]  # l  = bw   per p

_CACHE = {}


def _split_sync_waits(nc):
    """The staged walrus build accepts at most ONE sync wait per
    instruction, but Tile's wait-assignment freely attaches several.
    Hoist all but the last wait of each instruction onto same-engine
    NoOps inserted directly before it (engines execute their stream in
    order, so the semantics are identical)."""
    from concourse import mybir

    if getattr(nc, "_sync_waits_split", False):
        return
    nc._sync_waits_split = True
    counter = 0
    for bb in nc.m.functions[0].blocks:
        out = []
        changed = False
        for inst in bb.instructions:
            si = inst.sync_info
            waits = list(si.on_wait) if si is not None else []
            if len(waits) > 1:
                for w in waits[:-1]:
                    nop = mybir.InstNoOp(
                        name=f"I-splitw-{counter}", ins=[], outs=[])
                    counter += 1
                    nop.engine = inst.engine
                    nop.sync_info = mybir.SyncInfo(on_update=[], on_wait=[w])
                    out.append(nop)
                si.on_wait = waits[-1:]
                changed = True
            out.append(inst)
        if changed:
            bb.instructions = out


def _dct_matrix_f32() -> np.ndarray:
    k = np.arange(BS)[:, None].astype(np.float64)
    m = np.arange(BS)[None, :].astype(np.float64)
    D = np.cos(np.pi * (2.0 * m + 1.0) * k / (2.0 * BS)) * np.sqrt(2.0 / BS)
    D[0, :] = np.sqrt(1.0 / BS)
    return D.astype(np.float32)


def _canonical_indices():
    b, c, nh, nw, p = np.meshgrid(
        np.arange(B), np.arange(C), np.arange(NBH), np.arange(NBW),
        np.arange(NPOS), indexing="ij")
    return {
        "b_idx": b.reshape(-1).astype(np.int32),
        "c_idx": c.reshape(-1).astype(np.int32),
        "nh_idx": nh.reshape(-1).astype(np.int32),
        "nw_idx": nw.reshape(-1).astype(np.int32),
        "bh_idx": POS[p.reshape(-1), 0].astype(np.int32),
        "bw_idx": POS[p.reshape(-1), 1].astype(np.int32),
    }


def _is_canonical(b_idx, c_idx, nh_idx, nw_idx, bh_idx, bw_idx) -> bool:
    if b_idx.shape[0] != NUM_BITS:
        return False
    canon = _CACHE.setdefault("canon", _canonical_indices())
    got = {"b_idx": b_idx, "c_idx": c_idx, "nh_idx": nh_idx,
           "nw_idx": nw_idx, "bh_idx": bh_idx, "bw_idx": bw_idx}
    return all(np.array_equal(np.asarray(got[k]), canon[k]) for k in canon)


def _build_w_fast():
    """Wk [128 (q,j), 64 (q,plane)] bf16 hi/lo pair, packed [128, 1024]
    = [Wa(8k x 64) || Wb(8k x 64)]."""
    D = _dct_matrix_f32()
    Wt = np.zeros((128, 8, 64), dtype=np.float32)
    for k in range(8):
        for q in range(16):
            for j in range(8):
                for p in range(4):
                    Wt[q * 8 + j, k, q * 4 + p] = (
                        D[1 + IP[p], j] * D[LP[p], k])
    Wa = Wt.astype(ml_dtypes.bfloat16)
    Wb = (Wt - Wa.astype(np.float32)).astype(ml_dtypes.bfloat16)
    return np.concatenate(
        [Wa.reshape(128, 512), Wb.reshape(128, 512)], axis=1)


def build_fast_nc():
    """Per-core program: xa, xb [3, 128, 8192] bf16 -> po [3, 64, 1024]
    bf16 parity bits, po[c][q*4+p][g*128+nw]."""
    import concourse.bass as bass
    import concourse.tile as tile
    from concourse import mybir

    f32 = mybir.dt.float32
    bf = mybir.dt.bfloat16
    nc = bass.Bass()
    xa = nc.dram_tensor("xa", [C, 128, 8192], bf, kind="ExternalInput")
    xb = nc.dram_tensor("xb", [C, 128, 8192], bf, kind="ExternalInput")
    w = nc.dram_tensor("w", [128, 1024], bf, kind="ExternalInput")
    po = nc.dram_tensor("po", [C, 64, 1024], bf, kind="ExternalOutput")

    add = mybir.AluOpType.add
    mult = mybir.AluOpType.mult

    def chain(eng, src, tmp, par_dst, first_eng=None):
        """parity = |RNE(src)| mod 2 -> par_dst (bf16).  Rounding happens
        only at stored outputs; every intermediate is exactly
        representable (see module docstring).  first_eng handles the
        PSUM read (GpSimd cannot access PSUM)."""
        p = src.partition_size()
        n = src.free_size()
        t = tmp.tile([p, n], f32, tag="t")
        u = tmp.tile([p, n], f32, tag="u")
        (first_eng or eng).tensor_scalar(
            out=t[:], in0=src, scalar1=M15, scalar2=None, op0=add)
        eng.tensor_scalar(out=u[:], in0=t[:], scalar1=0.5,
                          scalar2=M15 / 2.0, op0=mult, op1=add)
        eng.tensor_scalar(out=u[:], in0=u[:], scalar1=-2.0, scalar2=M15,
                          op0=mult, op1=add)
        eng.tensor_tensor(out=t[:], in0=t[:], in1=u[:], op=add)
        nc.scalar.activation(
            out=par_dst, in_=t[:], func=mybir.ActivationFunctionType.Abs)

    with tile.TileContext(nc) as tc:
        with (
            tc.tile_pool(name="consts", bufs=1) as consts,
            tc.tile_pool(name="xs", bufs=2) as xpool,
            tc.tile_pool(name="tmp", bufs=4) as tmp,
            tc.tile_pool(name="par", bufs=1) as parpool,
            tc.tile_pool(name="psA", bufs=1, space="PSUM") as psA,
            tc.tile_pool(name="psB", bufs=1, space="PSUM") as psB,
        ):
            wt = consts.tile([128, 1024], bf)
            nc.sync.dma_start(out=wt[:], in_=w[:, :])

            accA = psA.tile([128, 1024], f32)  # c0 rows 0:64, c1 rows 64:
            accB = psB.tile([64, 1024], f32)   # c2
            parA = parpool.tile([128, 1024], bf)
            parB = parpool.tile([64, 1024], bf)

            for c in range(C):
                xat = xpool.tile([128, 8192], bf, tag="xa")
                xbt = xpool.tile([128, 8192], bf, tag="xb")
                # halves so the first matmul group starts sooner
                for h in range(2):
                    sl = slice(h * 4096, (h + 1) * 4096)
                    nc.sync.dma_start(out=xat[:, sl], in_=xa[c][:, sl])
                    nc.sync.dma_start(out=xbt[:, sl], in_=xb[c][:, sl])
                xav = xat[:].rearrange("p (g nw k) -> p g nw k", g=8, k=8)
                xbv = xbt[:].rearrange("p (g nw k) -> p g nw k", g=8, k=8)
                if c < 2:
                    outs = accA[c * 64:(c + 1) * 64, :]
                else:
                    outs = accB[:, :]
                for gh in range(2):
                    o = outs[:, gh * 512:(gh + 1) * 512]
                    idx = 0
                    for k in range(8):
                        wa = wt[:, k * 64:(k + 1) * 64]
                        wb = wt[:, 512 + k * 64:512 + (k + 1) * 64]
                        for lhsT, xv in ((wa, xav), (wa, xbv), (wb, xav)):
                            nc.tensor.matmul(
                                out=o, lhsT=lhsT,
                                rhs=xv[:, gh * 4:(gh + 1) * 4, :, k],
                                start=(idx == 0), stop=(idx == 23))
                            idx += 1
                if c == 1:
                    # c0+c1 parity on DVE while c2's matmuls run
                    for gh in range(2):
                        sl = slice(gh * 512, (gh + 1) * 512)
                        chain(nc.vector, accA[:, sl], tmp, parA[:, sl])
                if c == 2:
                    # split the tail across DVE and GpSimd
                    chain(nc.vector, accB[:, 0:512], tmp, parB[:, 0:512])
                    chain(nc.gpsimd, accB[:, 512:1024], tmp,
                          parB[:, 512:1024], first_eng=nc.vector)

            nc.sync.dma_start(out=po[0], in_=parA[0:64, :])
            nc.sync.dma_start(out=po[1], in_=parA[64:128, :])
            nc.sync.dma_start(out=po[2], in_=parB[:, :])
    return nc


def _prep_core(x):
    """x [3,1024,1024] f32 -> (xa, xb) [3, 128, 8192] bf16 hi/lo in the
    (q,j)-partition layout."""
    xa = x.astype(ml_dtypes.bfloat16)
    xb = (x - xa.astype(np.float32)).astype(ml_dtypes.bfloat16)

    def rl(a):
        a6 = a.reshape(C, 8, 16, 8, 128, 8).transpose(0, 2, 3, 1, 4, 5)
        return np.ascontiguousarray(a6).reshape(C, 128, 8192)

    return rl(xa), rl(xb)


def _build_consts_general():
    D = _dct_matrix_f32()
    BR8 = np.zeros((128, 128), dtype=np.float32)
    for nhl in range(16):
        BR8[nhl * 8:(nhl + 1) * 8, nhl * 8:(nhl + 1) * 8] = D.T  # [j, i]
    BC8 = np.zeros((128, 128), dtype=np.float32)
    for l in range(8):
        for nwl in range(16):
            BC8[nwl * 8:(nwl + 1) * 8, l * 16 + nwl] = D[l, :]
    return BR8, BC8


def _parity_ops_general(nc, pk, hk):
    """pk holds |coeff| >= 0; parity via 2^23 magic (each step its own
    instruction so every intermediate is rounded f32)."""
    from concourse import mybir

    ts = nc.vector.tensor_scalar
    add, sub, mult = (mybir.AluOpType.add, mybir.AluOpType.subtract,
                      mybir.AluOpType.mult)
    ts(out=pk[:], in0=pk[:], scalar1=MAGIC, scalar2=None, op0=add)
    ts(out=pk[:], in0=pk[:], scalar1=MAGIC, scalar2=None, op0=sub)
    ts(out=hk[:], in0=pk[:], scalar1=0.5, scalar2=None, op0=mult)
    ts(out=pk[:], in0=hk[:], scalar1=MAGIC, scalar2=None, op0=add)
    ts(out=pk[:], in0=pk[:], scalar1=MAGIC, scalar2=None, op0=sub)
    nc.vector.tensor_tensor(out=pk[:], in0=hk[:], in1=pk[:], op=sub)
    nc.scalar.activation(
        out=pk[:], in_=pk[:], func=mybir.ActivationFunctionType.Abs,
        scale=2.0)


def build_general_nc(nstrip=C * (H // 128)):
    """Per-core program: full 64-plane parity table (see baseline)."""
    import concourse.bass as bass
    import concourse.tile as tile
    from concourse import mybir

    f32 = mybir.dt.float32
    nc = bass.Bass()
    x = nc.dram_tensor("x", [C, H, W], f32, kind="ExternalInput")
    br = nc.dram_tensor("br", [128, 128], f32, kind="ExternalInput")
    bc = nc.dram_tensor("bc", [128, 128], f32, kind="ExternalInput")
    o = nc.dram_tensor("o", [nstrip, 128, 1024], f32, kind="ExternalOutput")

    with tile.TileContext(nc) as tc:
        with (
            tc.tile_pool(name="consts", bufs=1) as consts,
            tc.tile_pool(name="xs", bufs=2) as xpool,
            tc.tile_pool(name="ysb", bufs=2) as ypool,
            tc.tile_pool(name="pk", bufs=2) as pkpool,
            tc.tile_pool(name="yp", bufs=4, space="PSUM") as yppool,
            tc.tile_pool(name="fp", bufs=4, space="PSUM") as fppool,
        ):
            brt = consts.tile([128, 128], f32)
            nc.sync.dma_start(out=brt[:], in_=br[:, :])
            bct = consts.tile([128, 128], f32)
            nc.sync.dma_start(out=bct[:], in_=bc[:, :])

            for s in range(nstrip):
                c, hg = divmod(s, H // 128)
                xs = xpool.tile([128, 1024], f32, tag="xs")
                nc.sync.dma_start(
                    out=xs[:], in_=x[c, hg * 128:(hg + 1) * 128, :])
                ysb = ypool.tile([128, 1024], f32, tag="ysb")
                for wc in range(8):
                    yp = yppool.tile([128, 128], f32, tag="yp")
                    nc.tensor.matmul(
                        out=yp[:],
                        lhsT=xs[:, wc * 128:(wc + 1) * 128],
                        rhs=brt[:],
                        start=True, stop=True)
                    nc.vector.tensor_copy(
                        out=ysb[:, wc * 128:(wc + 1) * 128], in_=yp[:])
                pk = pkpool.tile([128, 1024], f32, tag="pk")
                hk = pkpool.tile([128, 1024], f32, tag="hk")
                for wc in range(8):
                    fp = fppool.tile([128, 128], f32, tag="fp")
                    nc.tensor.matmul(
                        out=fp[:],
                        lhsT=bct[:],
                        rhs=ysb[:, wc * 128:(wc + 1) * 128],
                        start=True, stop=True)
                    nc.scalar.activation(
                        out=pk[:, wc * 128:(wc + 1) * 128], in_=fp[:],
                        func=mybir.ActivationFunctionType.Abs)
                _parity_ops_general(nc, pk, hk)
                nc.sync.dma_start(out=o[s], in_=pk[:])
    return nc


def _run_spmd(nc, in_maps, trace=False):
    from concourse.bass_utils import run_bass_kernel_spmd

    _split_sync_waits(nc)

    res = run_bass_kernel_spmd(
        nc, in_maps, core_ids=list(range(B)), trace=trace)
    _CACHE["last_results"] = res
    return res.results


def _fast_path(stego, trace=False):
    key = "fast_nc2"
    if key not in _CACHE:
        _CACHE[key] = build_fast_nc()
    nc = _CACHE[key]
    Wp = _CACHE.setdefault("consts_fast2", _build_w_fast())
    in_maps = []
    for b in range(B):
        xa, xb = _prep_core(stego[b])
        in_maps.append({"xa": xa, "xb": xb, "w": Wp})
    results = _run_spmd(nc, in_maps, trace=trace)
    out = np.zeros((B, NUM_BITS), dtype=np.float32)
    for b in range(B):
        po = results[b]["po"].astype(np.float32)  # [3, 64, 1024]
        seg = np.ascontiguousarray(
            po.reshape(C, 16, 4, 8, 128).transpose(0, 3, 1, 4, 2)
        ).reshape(-1)
        out[b, b * SEG:(b + 1) * SEG] = seg
    return out


def _general_path(stego, b_idx, c_idx, nh_idx, nw_idx, bh_idx, bw_idx,
                  trace=False):
    key = "general_nc"
    if key not in _CACHE:
        _CACHE[key] = build_general_nc()
    nc = _CACHE[key]
    BR8, BC8 = _CACHE.setdefault("consts_general", _build_consts_general())
    in_maps = [
        {"x": np.ascontiguousarray(stego[b]), "br": BR8, "bc": BC8}
        for b in range(B)
    ]
    results = _run_spmd(nc, in_maps, trace=trace)

    b_idx = np.asarray(b_idx).astype(np.int64)
    c_idx = np.asarray(c_idx).astype(np.int64)
    nh_idx = np.asarray(nh_idx).astype(np.int64)
    nw_idx = np.asarray(nw_idx).astype(np.int64)
    bh_idx = np.asarray(bh_idx).astype(np.int64)
    bw_idx = np.asarray(bw_idx).astype(np.int64)
    num_bits = b_idx.shape[0]

    # table[s=(c,hg), l*16+nwl, wc*128 + nhl*8 + i]
    s = c_idx * 8 + nh_idx // 16
    part = bw_idx * 16 + nw_idx % 16
    free = (nw_idx // 16) * 128 + (nh_idx % 16) * 8 + bh_idx
    flat = (s * 128 + part) * 1024 + free

    out = np.zeros((B, num_bits), dtype=np.float32)
    cols = np.arange(num_bits)
    for b in range(B):
        tb = results[b]["o"].reshape(-1)
        mask = b_idx == b
        out[b, cols[mask]] = tb[flat[mask]]
    return out


def kernel(stego, b_idx, c_idx, nh_idx, nw_idx, bh_idx, bw_idx):
    stego = np.ascontiguousarray(np.asarray(stego, dtype=np.float32))
    import os
    trace = os.environ.get("BASS_TRACE", "") not in ("", "0")
    if _is_canonical(b_idx, c_idx, nh_idx, nw_idx, bh_idx, bw_idx):
        return _fast_path(stego, trace=trace)
    return _general_path(
        stego, b_idx, c_idx, nh_idx, nw_idx, bh_idx, bw_idx, trace=trace)


# revision 6
# speedup vs baseline: 2.2001x; 2.2001x over previous
"""Trainium2 Bass kernel for nn_DCTExtractor.

Reference computation:
  - stego [8, 3, 1024, 1024] f32; per 8x8 block 2D DCT-II (D @ X @ D^T).
  - bits[i] = abs(round_half_even(dct[b,c,nh,nw,bh,bw])) % 2 for 1572864
    index tuples.
  - out [8, num_bits]: out[b_idx[i], i] = bits[i]; other rows 0.

Sharding: data-parallel over batch b across the 8 NeuronCores; core b
processes image b and produces output row b.

Fast path (canonical meshgrid indices): each needed coefficient is a
Frobenius inner product <X_block, D_bh (x) D_bw>.  The host re-lays each
image out with BOTH within-block axes on partitions:

    x[c, (q, j) 128, (g, nw, k) 8192]   with nh = g*16 + q, w = nw*8 + k

and splits it into a bf16 hi/lo pair (xa + xb == x to ~16 mantissa bits).
The whole per-block 2D DCT then becomes 8 PSUM-accumulated bf16 matmuls
per (channel, g-half) with CONSTANT stationaries

    Wk[(q,j), (q,plane)] = D[bh_p, j] * D[bw_p, k]   (block-diagonal in q)

run as 3 split terms (Wa@xa + Wa@xb + Wb@xa) for fp32-grade accuracy:
every pixel streams through the PE exactly 3 times at 1 cycle/row and the
coefficients land already organized [64 (q,plane), (g,nw)] for the DVE
parity chain (round-to-int via the 1.5*2^23 magic trick, all
intermediates exact).  Parity bits return as bf16 (exact for 0/1).

General fallback (arbitrary indices): device computes the full 64-plane
parity table per image; host gathers bits and applies the b mask.
"""

import sys

if "/opt/trn_rl_repo" not in sys.path:
    sys.path.insert(0, "/opt/trn_rl_repo")

import numpy as np
import ml_dtypes

BS = 8
B, C, H, W = 8, 3, 1024, 1024
NBH, NBW = H // BS, W // BS
POS = np.array([[1, 2], [2, 1], [2, 2], [3, 1]], dtype=np.int32)
NPOS = 4
SEG = C * NBH * NBW * NPOS  # bits per batch element = 196608
NUM_BITS = B * SEG
MAGIC = float(np.float32(8388608.0))  # 2^23 (general path, abs first)
M15 = float(np.float32(12582912.0))  # 1.5*2^23: RNE for signed values
IP = [0, 1, 1, 2]  # i' = bh-1 per p
LP = [2, 1, 2, 1]  # l  = bw   per p

_CACHE = {}


def _split_sync_waits(nc):
    """The staged walrus build accepts at most ONE sync wait per
    instruction, but Tile's wait-assignment freely attaches several.
    Hoist all but the last wait of each instruction onto same-engine
    NoOps inserted directly before it (engines execute their stream in
    order, so the semantics are identical)."""
    from concourse import mybir

    if getattr(nc, "_sync_waits_split", False):
        return
    nc._sync_waits_split = True
    counter = 0
    for bb in nc.m.functions[0].blocks:
        out = []
        changed = False
        for inst in bb.instructions:
            si = inst.sync_info
            waits = list(si.on_wait) if si is not None else []
            if len(waits) > 1:
                for w in waits[:-1]:
                    nop = mybir.InstNoOp(
                        name=f"I-splitw-{counter}", ins=[], outs=[])
                    counter += 1
                    nop.engine = inst.engine
                    nop.sync_info = mybir.SyncInfo(on_update=[], on_wait=[w])
                    out.append(nop)
                si.on_wait = waits[-1:]
                changed = True
            out.append(inst)
        if changed:
            bb.instructions = out


def _dct_matrix_f32() -> np.ndarray:
    k = np.arange(BS)[:, None].astype(np.float64)
    m = np.arange(BS)[None, :].astype(np.float64)
    D = np.cos(np.pi * (2.0 * m + 1.0) * k / (2.0 * BS)) * np.sqrt(2.0 / BS)
    D[0, :] = np.sqrt(1.0 / BS)
    return D.astype(np.float32)


def _canonical_indices():
    b, c, nh, nw, p = np.meshgrid(
        np.arange(B), np.arange(C), np.arange(NBH), np.arange(NBW),
        np.arange(NPOS), indexing="ij")
    return {
        "b_idx": b.reshape(-1).astype(np.int32),
        "c_idx": c.reshape(-1).astype(np.int32),
        "nh_idx": nh.reshape(-1).astype(np.int32),
        "nw_idx": nw.reshape(-1).astype(np.int32),
        "bh_idx": POS[p.reshape(-1), 0].astype(np.int32),
        "bw_idx": POS[p.reshape(-1), 1].astype(np.int32),
    }


def _is_canonical(b_idx, c_idx, nh_idx, nw_idx, bh_idx, bw_idx) -> bool:
    if b_idx.shape[0] != NUM_BITS:
        return False
    canon = _CACHE.setdefault("canon", _canonical_indices())
    got = {"b_idx": b_idx, "c_idx": c_idx, "nh_idx": nh_idx,
           "nw_idx": nw_idx, "bh_idx": bh_idx, "bw_idx": bw_idx}
    return all(np.array_equal(np.asarray(got[k]), canon[k]) for k in canon)


def _build_w_fast():
    """Wk [128 (q,j), 64 (q,plane)] bf16 hi/lo pair, packed [128, 1024]
    = [Wa(8k x 64) || Wb(8k x 64)]."""
    D = _dct_matrix_f32()
    Wt = np.zeros((128, 8, 64), dtype=np.float32)
    for k in range(8):
        for q in range(16):
            for j in range(8):
                for p in range(4):
                    Wt[q * 8 + j, k, q * 4 + p] = (
                        D[1 + IP[p], j] * D[LP[p], k])
    Wa = Wt.astype(ml_dtypes.bfloat16)
    Wb = (Wt - Wa.astype(np.float32)).astype(ml_dtypes.bfloat16)
    return np.concatenate(
        [Wa.reshape(128, 512), Wb.reshape(128, 512)], axis=1)


def build_fast_nc():
    """Per-core program: xa, xb [3, 128, 8192] bf16 -> po [3, 64, 1024]
    bf16 parity bits, po[c][q*4+p][g*128+nw]."""
    import concourse.bass as bass
    import concourse.tile as tile
    from concourse import mybir

    f32 = mybir.dt.float32
    bf = mybir.dt.bfloat16
    nc = bass.Bass()
    xa = nc.dram_tensor("xa", [C, 128, 8192], bf, kind="ExternalInput")
    xb = nc.dram_tensor("xb", [C, 128, 8192], bf, kind="ExternalInput")
    w = nc.dram_tensor("w", [128, 1024], bf, kind="ExternalInput")
    po = nc.dram_tensor("po", [C, 64, 1024], bf, kind="ExternalOutput")

    add = mybir.AluOpType.add
    mult = mybir.AluOpType.mult

    def chain(eng, src, tmp, par_dst, first_eng=None):
        """parity = |RNE(src)| mod 2 -> par_dst (bf16).  Rounding happens
        only at stored outputs; every intermediate is exactly
        representable (see module docstring).  first_eng handles the
        PSUM read (GpSimd cannot access PSUM)."""
        p = src.partition_size()
        n = src.free_size()
        t = tmp.tile([p, n], f32, tag="t")
        u = tmp.tile([p, n], f32, tag="u")
        (first_eng or eng).tensor_scalar(
            out=t[:], in0=src, scalar1=M15, scalar2=None, op0=add)
        eng.tensor_scalar(out=u[:], in0=t[:], scalar1=0.5,
                          scalar2=M15 / 2.0, op0=mult, op1=add)
        eng.tensor_scalar(out=u[:], in0=u[:], scalar1=-2.0, scalar2=M15,
                          op0=mult, op1=add)
        eng.tensor_tensor(out=t[:], in0=t[:], in1=u[:], op=add)
        nc.scalar.activation(
            out=par_dst, in_=t[:], func=mybir.ActivationFunctionType.Abs)

    with tile.TileContext(nc) as tc:
        with (
            tc.tile_pool(name="consts", bufs=1) as consts,
            tc.tile_pool(name="xs", bufs=2) as xpool,
            tc.tile_pool(name="tmp", bufs=4) as tmp,
            tc.tile_pool(name="par", bufs=1) as parpool,
            tc.tile_pool(name="psA", bufs=1, space="PSUM") as psA,
            tc.tile_pool(name="psB", bufs=1, space="PSUM") as psB,
        ):
            wt = consts.tile([128, 1024], bf)
            nc.sync.dma_start(out=wt[:], in_=w[:, :])

            accA = psA.tile([128, 1024], f32)  # c0 rows 0:64, c1 rows 64:
            accB = psB.tile([64, 1024], f32)   # c2
            parA = parpool.tile([128, 1024], bf)
            parB = parpool.tile([64, 1024], bf)

            for c in range(C):
                xat = xpool.tile([128, 8192], bf, tag="xa")
                xbt = xpool.tile([128, 8192], bf, tag="xb")
                # halves so the first matmul group starts sooner
                for h in range(2):
                    sl = slice(h * 4096, (h + 1) * 4096)
                    nc.sync.dma_start(out=xat[:, sl], in_=xa[c][:, sl])
                    nc.sync.dma_start(out=xbt[:, sl], in_=xb[c][:, sl])
                # free layout (g, k, nw): per-matmul columns contiguous
                xav = xat[:].rearrange("p (g k nw) -> p g k nw", g=8, k=8)
                xbv = xbt[:].rearrange("p (g k nw) -> p g k nw", g=8, k=8)
                if c < 2:
                    outs = accA[c * 64:(c + 1) * 64, :]
                else:
                    outs = accB[:, :]
                for gh in range(2):
                    o = outs[:, gh * 512:(gh + 1) * 512]
                    idx = 0
                    for k in range(8):
                        wa = wt[:, k * 64:(k + 1) * 64]
                        wb = wt[:, 512 + k * 64:512 + (k + 1) * 64]
                        for lhsT, xv in ((wa, xav), (wa, xbv), (wb, xav)):
                            nc.tensor.matmul(
                                out=o, lhsT=lhsT,
                                rhs=xv[:, gh * 4:(gh + 1) * 4, k, :],
                                start=(idx == 0), stop=(idx == 23))
                            idx += 1
                if c == 1:
                    # c0+c1 parity on DVE while c2's matmuls run
                    for gh in range(2):
                        sl = slice(gh * 512, (gh + 1) * 512)
                        chain(nc.vector, accA[:, sl], tmp, parA[:, sl])
                if c == 2:
                    # split the tail across DVE and GpSimd
                    chain(nc.vector, accB[:, 0:512], tmp, parB[:, 0:512])
                    chain(nc.gpsimd, accB[:, 512:1024], tmp,
                          parB[:, 512:1024], first_eng=nc.vector)

            nc.sync.dma_start(out=po[0], in_=parA[0:64, :])
            nc.sync.dma_start(out=po[1], in_=parA[64:128, :])
            nc.sync.dma_start(out=po[2], in_=parB[:, :])
    return nc


def _prep_core(x):
    """x [3,1024,1024] f32 -> (xa, xb) [3, 128, 8192] bf16 hi/lo in the
    (q,j)-partition layout."""
    xa = x.astype(ml_dtypes.bfloat16)
    xb = (x - xa.astype(np.float32)).astype(ml_dtypes.bfloat16)

    def rl(a):
        a6 = a.reshape(C, 8, 16, 8, 128, 8).transpose(0, 2, 3, 1, 5, 4)
        return np.ascontiguousarray(a6).reshape(C, 128, 8192)

    return rl(xa), rl(xb)


def _build_consts_general():
    D = _dct_matrix_f32()
    BR8 = np.zeros((128, 128), dtype=np.float32)
    for nhl in range(16):
        BR8[nhl * 8:(nhl + 1) * 8, nhl * 8:(nhl + 1) * 8] = D.T  # [j, i]
    BC8 = np.zeros((128, 128), dtype=np.float32)
    for l in range(8):
        for nwl in range(16):
            BC8[nwl * 8:(nwl + 1) * 8, l * 16 + nwl] = D[l, :]
    return BR8, BC8


def _parity_ops_general(nc, pk, hk):
    """pk holds |coeff| >= 0; parity via 2^23 magic (each step its own
    instruction so every intermediate is rounded f32)."""
    from concourse import mybir

    ts = nc.vector.tensor_scalar
    add, sub, mult = (mybir.AluOpType.add, mybir.AluOpType.subtract,
                      mybir.AluOpType.mult)
    ts(out=pk[:], in0=pk[:], scalar1=MAGIC, scalar2=None, op0=add)
    ts(out=pk[:], in0=pk[:], scalar1=MAGIC, scalar2=None, op0=sub)
    ts(out=hk[:], in0=pk[:], scalar1=0.5, scalar2=None, op0=mult)
    ts(out=pk[:], in0=hk[:], scalar1=MAGIC, scalar2=None, op0=add)
    ts(out=pk[:], in0=pk[:], scalar1=MAGIC, scalar2=None, op0=sub)
    nc.vector.tensor_tensor(out=pk[:], in0=hk[:], in1=pk[:], op=sub)
    nc.scalar.activation(
        out=pk[:], in_=pk[:], func=mybir.ActivationFunctionType.Abs,
        scale=2.0)


def build_general_nc(nstrip=C * (H // 128)):
    """Per-core program: full 64-plane parity table (see baseline)."""
    import concourse.bass as bass
    import concourse.tile as tile
    from concourse import mybir

    f32 = mybir.dt.float32
    nc = bass.Bass()
    x = nc.dram_tensor("x", [C, H, W], f32, kind="ExternalInput")
    br = nc.dram_tensor("br", [128, 128], f32, kind="ExternalInput")
    bc = nc.dram_tensor("bc", [128, 128], f32, kind="ExternalInput")
    o = nc.dram_tensor("o", [nstrip, 128, 1024], f32, kind="ExternalOutput")

    with tile.TileContext(nc) as tc:
        with (
            tc.tile_pool(name="consts", bufs=1) as consts,
            tc.tile_pool(name="xs", bufs=2) as xpool,
            tc.tile_pool(name="ysb", bufs=2) as ypool,
            tc.tile_pool(name="pk", bufs=2) as pkpool,
            tc.tile_pool(name="yp", bufs=4, space="PSUM") as yppool,
            tc.tile_pool(name="fp", bufs=4, space="PSUM") as fppool,
        ):
            brt = consts.tile([128, 128], f32)
            nc.sync.dma_start(out=brt[:], in_=br[:, :])
            bct = consts.tile([128, 128], f32)
            nc.sync.dma_start(out=bct[:], in_=bc[:, :])

            for s in range(nstrip):
                c, hg = divmod(s, H // 128)
                xs = xpool.tile([128, 1024], f32, tag="xs")
                nc.sync.dma_start(
                    out=xs[:], in_=x[c, hg * 128:(hg + 1) * 128, :])
                ysb = ypool.tile([128, 1024], f32, tag="ysb")
                for wc in range(8):
                    yp = yppool.tile([128, 128], f32, tag="yp")
                    nc.tensor.matmul(
                        out=yp[:],
                        lhsT=xs[:, wc * 128:(wc + 1) * 128],
                        rhs=brt[:],
                        start=True, stop=True)
                    nc.vector.tensor_copy(
                        out=ysb[:, wc * 128:(wc + 1) * 128], in_=yp[:])
                pk = pkpool.tile([128, 1024], f32, tag="pk")
                hk = pkpool.tile([128, 1024], f32, tag="hk")
                for wc in range(8):
                    fp = fppool.tile([128, 128], f32, tag="fp")
                    nc.tensor.matmul(
                        out=fp[:],
                        lhsT=bct[:],
                        rhs=ysb[:, wc * 128:(wc + 1) * 128],
                        start=True, stop=True)
                    nc.scalar.activation(
                        out=pk[:, wc * 128:(wc + 1) * 128], in_=fp[:],
                        func=mybir.ActivationFunctionType.Abs)
                _parity_ops_general(nc, pk, hk)
                nc.sync.dma_start(out=o[s], in_=pk[:])
    return nc


def _run_spmd(nc, in_maps, trace=False):
    from concourse.bass_utils import run_bass_kernel_spmd

    _split_sync_waits(nc)

    res = run_bass_kernel_spmd(
        nc, in_maps, core_ids=list(range(B)), trace=trace)
    _CACHE["last_results"] = res
    return res.results


def _fast_path(stego, trace=False):
    key = "fast_nc2"
    if key not in _CACHE:
        _CACHE[key] = build_fast_nc()
    nc = _CACHE[key]
    Wp = _CACHE.setdefault("consts_fast2", _build_w_fast())
    in_maps = []
    for b in range(B):
        xa, xb = _prep_core(stego[b])
        in_maps.append({"xa": xa, "xb": xb, "w": Wp})
    results = _run_spmd(nc, in_maps, trace=trace)
    out = np.zeros((B, NUM_BITS), dtype=np.float32)
    for b in range(B):
        po = results[b]["po"].astype(np.float32)  # [3, 64, 1024]
        seg = np.ascontiguousarray(
            po.reshape(C, 16, 4, 8, 128).transpose(0, 3, 1, 4, 2)
        ).reshape(-1)
        out[b, b * SEG:(b + 1) * SEG] = seg
    return out


def _general_path(stego, b_idx, c_idx, nh_idx, nw_idx, bh_idx, bw_idx,
                  trace=False):
    key = "general_nc"
    if key not in _CACHE:
        _CACHE[key] = build_general_nc()
    nc = _CACHE[key]
    BR8, BC8 = _CACHE.setdefault("consts_general", _build_consts_general())
    in_maps = [
        {"x": np.ascontiguousarray(stego[b]), "br": BR8, "bc": BC8}
        for b in range(B)
    ]
    results = _run_spmd(nc, in_maps, trace=trace)

    b_idx = np.asarray(b_idx).astype(np.int64)
    c_idx = np.asarray(c_idx).astype(np.int64)
    nh_idx = np.asarray(nh_idx).astype(np.int64)
    nw_idx = np.asarray(nw_idx).astype(np.int64)
    bh_idx = np.asarray(bh_idx).astype(np.int64)
    bw_idx = np.asarray(bw_idx).astype(np.int64)
    num_bits = b_idx.shape[0]

    # table[s=(c,hg), l*16+nwl, wc*128 + nhl*8 + i]
    s = c_idx * 8 + nh_idx // 16
    part = bw_idx * 16 + nw_idx % 16
    free = (nw_idx // 16) * 128 + (nh_idx % 16) * 8 + bh_idx
    flat = (s * 128 + part) * 1024 + free

    out = np.zeros((B, num_bits), dtype=np.float32)
    cols = np.arange(num_bits)
    for b in range(B):
        tb = results[b]["o"].reshape(-1)
        mask = b_idx == b
        out[b, cols[mask]] = tb[flat[mask]]
    return out


def kernel(stego, b_idx, c_idx, nh_idx, nw_idx, bh_idx, bw_idx):
    stego = np.ascontiguousarray(np.asarray(stego, dtype=np.float32))
    import os
    trace = os.environ.get("BASS_TRACE", "") not in ("", "0")
    if _is_canonical(b_idx, c_idx, nh_idx, nw_idx, bh_idx, bw_idx):
        return _fast_path(stego, trace=trace)
    return _general_path(
        stego, b_idx, c_idx, nh_idx, nw_idx, bh_idx, bw_idx, trace=trace)
